# revision 18
# baseline (speedup 1.0000x reference)
"""Multi-head causal self-attention (SEQ=4096, D=1024, H=16, Dh=64) on 8
Trainium2 NeuronCores.

Sharding: tensor-parallel over heads - 2 heads per core. Each core computes
its heads' Q/K/V projections, causal attention, and its partial output
projection Y_c = O_c @ Wo[:, c]^T. The 8 partials are summed on the host
(mathematically the all-reduce); bo and the V-bias correction Wo@bv are
added there.

Device kernel, fully software-pipelined (matmuls bf16, fp32 PSUM):
  - one loop over 8 q-blocks (QB=512); during block qb's attention k-loop
    the kernel interleaves: QK projections + V projection of block qb+1,
    softmax-normalization of block qb-1, and the output projection of
    block qb-1. No separate phases, so PE and ACT overlap end to end.
  - S^T = K Q^T per (q-block, k-tile): K stationaries are zero-padded to
    128 contraction rows (KTz0 = [K0;0], KTz1 = [0;K1]) so every matmul
    runs in the untiled 128x128 PE mode - no 64-row-mode switches, which
    cost a ~130-160ns array drain per switch.
  - V is projected with x-chunk stationaries (out = x_tile^T @ Wv chunk,
    [128 seq, 128 hp]), which directly yields the AV-stationary k-major
    layout - no PE transposes (transpose mode switches) anywhere.
  - AV: stationary [V_h | ones] (M=65) so the matmul also accumulates the
    softmax row-sums; normalization = DVE reciprocal + gpsimd
    partition_broadcast + DVE multiplies (ACT runs exp only, one table).
  - diagonal k-tiles are trimmed: S/exp/mask/AV only touch columns
    q >= 128*j, saving PE cycles, exp elements and gpsimd mask work.
  - bk is dropped (softmax shift invariance), bv/bo folded into a host
    constant Wo@bv+bo, y is stored fp16 (halves the writeback traffic).

The causal mask input is not read: the reference mask is tril(ones) by
construction and the kernel hardcodes causality.
"""
import sys

if '/opt/trn_rl_repo' not in sys.path:
    sys.path.insert(0, '/opt/trn_rl_repo')

import numpy as np

import concourse.bass as bass
import concourse.mybir as mybir
import concourse.tile as tile
from concourse.bass_utils import run_bass_kernel_spmd

SEQ = 4096
D = 1024
N_CORES = 8
HP = 128          # head dims per core (2 heads x 64)
DH = 64
QB = 512          # q-block
KB = 128          # k-tile
NQB = SEQ // QB   # 8
NKT = SEQ // KB   # 32
NDC = D // 128    # 8 contraction chunks for the projections

F32 = mybir.dt.float32
F16 = mybir.dt.float16
BF16 = mybir.dt.bfloat16

_NC_CACHE = None


def _split_waits(nc):
    """This walrus build allows only one sync-wait per instruction for
    several ISA structs (self-loading matmuls, drains, DMAs, DVE ops).
    Offload extra waits onto single-wait EventSemaphores inserted
    immediately before, on the same engine."""
    n = 0
    for f in nc.m.functions:
        for b in f.blocks:
            insts = b.instructions  # live list
            i = 0
            while i < len(insts):
                inst = insts[i]
                tn = type(inst).__name__
                if tn != 'InstEventSemaphore':
                    si = inst.sync_info
                    waits = list(si.on_wait) if si and si.on_wait else []
                    if len(waits) > 1:
                        for j, w in enumerate(waits[:-1]):
                            ev = mybir.InstEventSemaphore(
                                name=f'mmwait-{n}-{j}-{inst.name}',
                                engine=inst.engine,
                                ins=[], outs=[],
                                sync_info=mybir.SyncInfo(
                                    on_wait=[w], on_update=[]),
                            )
                            insts.insert(i, ev)
                            i += 1
                        inst.sync_info = mybir.SyncInfo(
                            on_wait=[waits[-1]],
                            on_update=list(si.on_update or []))
                        n += 1
                i += 1
    return n


def _build_nc():
    nc = bass.Bass()
    # x pre-chunked and pre-cast to bf16 on host:
    # [qc, p, c, q] = x[qc*QB+q, c*128+p]
    xT = nc.dram_tensor('xT', [NQB, 128, NDC, QB], BF16, kind='ExternalInput')
    # W pre-chunked, bf16: [p, c, m] = W.T[c*128+p, m]
    wqT = nc.dram_tensor('wqT', [128, NDC, HP], BF16, kind='ExternalInput')
    wkT = nc.dram_tensor('wkT', [128, NDC, HP], BF16, kind='ExternalInput')
    wvT = nc.dram_tensor('wvT', [128, NDC, HP], BF16, kind='ExternalInput')
    bq = nc.dram_tensor('bq', [HP, 1], F32, kind='ExternalInput')
    woT = nc.dram_tensor('woT', [HP, D], BF16, kind='ExternalInput')
    y = nc.dram_tensor('y', [SEQ, D], F16, kind='ExternalOutput')

    ydma_engines = None  # set inside

    with tile.TileContext(nc) as tc:
        with tc.tile_pool(name='persist', bufs=1) as persist, \
             tc.tile_pool(name='dram', bufs=1, space='DRAM') as dpool, \
             tc.tile_pool(name='xb', bufs=3) as xbpool, \
             tc.tile_pool(name='ps', bufs=1, space='PSUM') as ps, \
             tc.tile_pool(name='p01s', bufs=4) as ppool, \
             tc.tile_pool(name='rbp', bufs=2) as rbpool, \
             tc.tile_pool(name='yp', bufs=3) as ypool:

            ydma_engines = [nc.sync]

            def load_chunk(qc, nsplit=2):
                xb = xbpool.tile([128, NDC, QB], BF16, tag='xb')
                step = NDC // nsplit
                for a in range(nsplit):
                    csl = bass.ts(a, step)
                    nc.sync.dma_start(out=xb[:, csl, :],
                                      in_=xT[qc, :, csl, :])
                return xb

            # ---- persistent SBUF state ----
            bq_sb = persist.tile([HP, 1], F32)
            nc.gpsimd.dma_start(out=bq_sb, in_=bq[:, :])
            wq_b = persist.tile([128, NDC, HP], BF16)
            wk_b = persist.tile([128, NDC, HP], BF16)
            wv_b = persist.tile([128, NDC, HP], BF16)
            wo_b = persist.tile([HP, D], BF16)
            for dram_w, btile in ((wqT, wq_b), (wkT, wk_b), (wvT, wv_b)):
                nc.gpsimd.dma_start(out=btile, in_=dram_w[:, :, :])
            nc.gpsimd.dma_start(out=wo_b, in_=woT[:, :])

            QT = persist.tile([HP, SEQ], BF16)
            KTz0 = persist.tile([128, SEQ], BF16)  # [K0; 0]
            KTz1 = persist.tile([128, SEQ], BF16)  # [0; K1]
            V_sb = persist.tile([128, NKT, 130], BF16)  # [k, kt, V0|1|V1|1]
            OT = persist.tile([HP, SEQ], BF16)
            rs_row = persist.tile([1, NQB, 2, QB], F32)
            rs_dr = dpool.tile([1, NQB, 2, QB], F32)
            rc_dr = dpool.tile([1, NQB, 2, QB], F32)

            # zero the dead halves of the padded K stationaries, set the
            # AV ones-columns (both written once; projections never touch
            # them again)
            nc.vector.memset(KTz0[64:128, :], 0.0)
            nc.vector.memset(KTz1[0:64, :], 0.0)
            nc.vector.memset(V_sb[:, :, 64:65], 1.0)
            nc.vector.memset(V_sb[:, :, 129:130], 1.0)

            xbufs = {}
            xbufs[0] = load_chunk(0, nsplit=8)
            xbufs[1] = load_chunk(1, nsplit=2)

            # ---- PE warmup during the first x DMA ----
            warm = ps.tile([128, 128], F32, tag='work')
            for i in range(48):
                nc.tensor.matmul(warm[0:HP, :], wq_b[:, 0, :],
                                 wk_b[:, 0, :], start=(i == 0),
                                 stop=(i == 47))

            # ---- building blocks ----
            def qk_mm_task(qkt_ps, xb, qc, d):
                st = (d == 0)
                sp = (d == NDC - 1)
                nc.tensor.matmul(qkt_ps[:, 0, :], wq_b[:, d, :],
                                 xb[:, d, :], start=st, stop=sp)
                nc.tensor.matmul(qkt_ps[:, 1, :], wk_b[:, d, :],
                                 xb[:, d, :], start=st, stop=sp)

            def qk_copy_task(qkt_ps, qc):
                qsl = bass.ts(qc, QB)
                nc.vector.tensor_scalar_add(QT[:, qsl], qkt_ps[:, 0, :],
                                            bq_sb[:, 0:1])
                nc.vector.tensor_copy(out=KTz0[0:64, qsl],
                                      in_=qkt_ps[0:64, 1, :])
                nc.vector.tensor_copy(out=KTz1[64:128, qsl],
                                      in_=qkt_ps[64:128, 1, :])

            def v_mm_task(v_ps, xb, qc, j, d):
                nc.tensor.matmul(v_ps[:, j, :], xb[:, d, bass.ts(j, 128)],
                                 wv_b[:, d, :], start=(d == 0),
                                 stop=(d == NDC - 1))

            def v_copy_task(v_ps, qc, j):
                kt = qc * (QB // 128) + j
                nc.vector.tensor_copy(out=V_sb[:, kt, 0:DH],
                                      in_=v_ps[:, j, 0:DH])
                nc.vector.tensor_copy(out=V_sb[:, kt, 65:65 + DH],
                                      in_=v_ps[:, j, DH:2 * DH])

            def norm_a(qb, o01):
                # phase A, at the next block's start: move the row-sums
                # (o01 row 64, the AV ones column) and the unnormalized
                # O^T out of PSUM so o01's slot frees quickly, and compute
                # the reciprocals in a partition-parallel [128, 2, 4]
                # layout via a DRAM scatter round-trip (a single-partition
                # DVE reciprocal takes 6.5us; this one is ~0.2us).
                qsl = bass.ts(qb, QB)
                nc.vector.tensor_copy(out=rs_row[0:1, qb, :, :],
                                      in_=o01[64:65, :, :])
                nc.vector.tensor_copy(out=OT[0:DH, qsl],
                                      in_=o01[0:DH, 0, :])
                nc.vector.tensor_copy(out=OT[DH:2 * DH, qsl],
                                      in_=o01[0:DH, 1, :])
                nc.sync.dma_start(out=rs_dr[0:1, qb, :, :],
                                  in_=rs_row[0:1, qb, :, :])
                rs_t = rbpool.tile([128, 2, 4], F32, tag='rst')
                nc.sync.dma_start(
                    out=rs_t,
                    in_=bass.AP(tensor=rs_dr.tensor,
                                offset=rs_dr.offset + 2 * qb * QB,
                                ap=[[1, 128], [QB, 2], [128, 4]]))
                rc_t = rbpool.tile([128, 2, 4], F32, tag='rct')
                nc.vector.reciprocal(out=rc_t, in_=rs_t)
                nc.sync.dma_start(
                    out=bass.AP(tensor=rc_dr.tensor,
                                offset=rc_dr.offset + 2 * qb * QB,
                                ap=[[1, 128], [QB, 2], [128, 4]]),
                    in_=rc_t)

            def norm_b(qb):
                # phase B, an interleaved task late in the next block:
                # broadcast the reciprocals across partitions (DMA read
                # with partition stride 0) and scale OT in place. Runs on
                # gpsimd+DVE only, after the block's affine_selects are
                # already enqueued.
                qsl = bass.ts(qb, QB)
                rb = rbpool.tile([128, QB], F32, tag='rb')
                for h in range(2):
                    nc.gpsimd.dma_start(
                        out=rb[h * DH:(h + 1) * DH, :],
                        in_=bass.AP(tensor=rc_dr.tensor,
                                    offset=rc_dr.offset + (2 * qb + h) * QB,
                                    ap=[[0, DH], [1, QB]]))
                nc.vector.tensor_mul(OT[0:DH, qsl],
                                     OT[0:DH, qsl], rb[0:DH, :])
                nc.vector.tensor_mul(OT[DH:2 * DH, qsl],
                                     OT[DH:2 * DH, qsl], rb[DH:2 * DH, :])

            ycount = [0]

            def yproj_task(t, tag='work'):
                # output projection of q-tile t (128 rows), from OT
                qt_sl = bass.ts(t, 128)
                y01 = ps.tile([128, 2, QB], F32, tag=tag,
                              bufs=2 if tag == 's01' else None)
                nc.tensor.matmul(y01[:, 0, :], OT[:, qt_sl],
                                 wo_b[:, 0:QB], start=True, stop=True)
                nc.tensor.matmul(y01[:, 1, :], OT[:, qt_sl],
                                 wo_b[:, QB:D], start=True, stop=True)
                ysb = ypool.tile([128, D], F16)
                nc.vector.tensor_copy(out=ysb,
                                      in_=y01.rearrange('p a b -> p (a b)'))
                eng = ydma_engines[ycount[0] % len(ydma_engines)]
                ycount[0] += 1
                eng.dma_start(out=y[qt_sl, :], in_=ysb)

            # ---- block-qb work queue (interleaved into the k-loop) ----
            def make_work(qb):
                tasks = []
                pj = qb + 1
                if pj < NQB:
                    qkt_ps = ps.tile([128, 2, QB], F32, tag='work',
                                     name=f'qkt{pj}')
                    xb = xbufs[pj]
                    for d in range(NDC):
                        tasks.append(lambda q=pj, dd=d, t=qkt_ps, x=xb:
                                     qk_mm_task(t, x, q, dd))
                    tasks.append(lambda q=pj, t=qkt_ps: qk_copy_task(t, q))
                    v_ps = ps.tile([128, 4, 128], F32, tag='work',
                                   name=f'vps{pj}')
                    for j in range(4):
                        for d in range(NDC):
                            tasks.append(lambda q=pj, jj=j, dd=d, t=v_ps,
                                         x=xb: v_mm_task(t, x, q, jj, dd))
                        tasks.append(lambda q=pj, jj=j, t=v_ps:
                                     v_copy_task(t, q, jj))
                    if pj + 1 < NQB:
                        # prefetch x for the next projection block
                        tasks.append(lambda q=pj + 1:
                                     xbufs.__setitem__(q, load_chunk(q)))
                if qb >= 1:
                    tasks.append(lambda b=qb - 1: norm_b(b))
                    for t in range((qb - 1) * 4, qb * 4):
                        tasks.append(lambda tt=t: yproj_task(tt))
                return tasks

            # ---- projection of block 0 (bootstrap, d-chunk streamed) ----
            qkt0 = ps.tile([128, 2, QB], F32, tag='work')
            for d in range(NDC):
                qk_mm_task(qkt0, xbufs[0], 0, d)
            qk_copy_task(qkt0, 0)
            v0 = ps.tile([128, 4, 128], F32, tag='work')
            for j in range(4):
                for d in range(NDC):
                    v_mm_task(v0, xbufs[0], 0, j, d)
                v_copy_task(v0, 0, j)

            # ---- main loop: attention(qb) + interleaved work ----
            prev_o01 = None
            for qb in range(NQB):
                qsl = bass.ts(qb, QB)
                nsteps = (qb + 1) * (QB // KB)
                diag0 = qb * (QB // KB)
                if qb >= 1:
                    # must precede the o01 reallocation below: the slot is
                    # single-buffered, so the norm reads of block qb-1 have
                    # to be in program order before block qb's first AV
                    norm_a(qb - 1, prev_o01)
                o01 = ps.tile([65, 2, QB], F32, tag='o01')
                tasks = make_work(qb)
                prev_o01 = o01
                # spread tasks over steps (front-loaded so proj finishes
                # before the next block needs it)
                ntask = len(tasks)
                emitted = 0
                pending_av = None
                for step in range(nsteps):
                    kt = step
                    j = kt - diag0
                    off = 128 * j if j >= 1 else 0
                    q0 = qb * QB + off
                    q1 = (qb + 1) * QB
                    s01 = ps.tile([128, 2, QB], F32, tag='s01', bufs=2)
                    ksl = bass.ts(kt, KB)
                    nc.tensor.matmul(s01[:, 0, off:], KTz0[:, ksl],
                                     QT[:, q0:q1], start=True, stop=True)
                    nc.tensor.matmul(s01[:, 1, off:], KTz1[:, ksl],
                                     QT[:, q0:q1], start=True, stop=True)
                    p01 = ppool.tile([128, 2, QB], BF16, tag='p01')
                    nc.scalar.activation(
                        out=p01[:, :, off:], in_=s01[:, :, off:],
                        func=mybir.ActivationFunctionType.Exp)
                    if j >= 0:
                        # diagonal tile: zero entries with k > q in the
                        # first 128 valid columns; later columns are
                        # always unmasked
                        nc.gpsimd.affine_select(
                            out=p01[:, :, off:off + 128],
                            in_=p01[:, :, off:off + 128],
                            compare_op=mybir.AluOpType.is_ge,
                            fill=0.0, base=0,
                            pattern=[[0, 2], [1, 128]],
                            channel_multiplier=-1)
                    # AV of the previous step: one step of lag keeps the
                    # in-order PE queue from stalling on exp(n)
                    if pending_av is not None:
                        pending_av()
                    st = (step == 0)
                    sp = (step == nsteps - 1)

                    def av(o=off, k=kt, p=p01, st=st, sp=sp):
                        nc.tensor.matmul(o01[:, 0, o:], V_sb[:, k, 0:65],
                                         p[:, 0, o:], start=st, stop=sp,
                                         skip_group_check=True)
                        nc.tensor.matmul(o01[:, 1, o:], V_sb[:, k, 65:130],
                                         p[:, 1, o:], start=st, stop=sp,
                                         skip_group_check=True)
                    pending_av = av
                    # interleave background work
                    want = (step + 1) * ntask // nsteps
                    while emitted < want:
                        tasks[emitted]()
                        emitted += 1
                pending_av()

            # ---- tail: last block's norm + output projection ----
            norm_a(NQB - 1, prev_o01)
            norm_b(NQB - 1)
            for t in range((NQB - 1) * 4, NQB * 4):
                yproj_task(t, tag='s01')

    _split_waits(nc)
    return nc


def get_nc():
    global _NC_CACHE
    if _NC_CACHE is None:
        _NC_CACHE = _build_nc()
    return _NC_CACHE


def _chunk_w(wT):
    # [D, HP] -> [p, c, m] with D = c*128 + p, cast to bf16
    import ml_dtypes
    return np.ascontiguousarray(
        wT.reshape(NDC, 128, HP).transpose(1, 0, 2)).astype(
            ml_dtypes.bfloat16)


def build_in_maps(inputs):
    import ml_dtypes
    x = np.asarray(inputs['x'], np.float32)
    # [qc, p, c, q] = x[qc*QB+q, c*128+p], bf16
    xc = np.ascontiguousarray(
        x.reshape(NQB, QB, NDC, 128).transpose(0, 3, 2, 1)).astype(
            ml_dtypes.bfloat16)
    scale = 1.0 / np.sqrt(DH)
    Wq = np.asarray(inputs['Wq'], np.float32)
    Wk = np.asarray(inputs['Wk'], np.float32)
    Wv = np.asarray(inputs['Wv'], np.float32)
    Wo = np.asarray(inputs['Wo'], np.float32)
    bq = np.asarray(inputs['bq'], np.float32)
    in_maps = []
    for c in range(N_CORES):
        sl = slice(c * HP, (c + 1) * HP)
        in_maps.append({
            'xT': xc,
            'wqT': _chunk_w((Wq[sl, :] * scale).T),
            'wkT': _chunk_w(Wk[sl, :].T),
            'wvT': _chunk_w(Wv[sl, :].T),
            'bq': np.ascontiguousarray((bq[sl] * scale).reshape(HP, 1)),
            'woT': np.ascontiguousarray(Wo[:, sl].T).astype(
                ml_dtypes.bfloat16),
        })
    return in_maps


def gather(results, inputs):
    y = np.zeros((SEQ, D), np.float32)
    for r in results:
        y += np.asarray(r['y'], np.float32)
    Wo = np.asarray(inputs['Wo'], np.float32)
    bv = np.asarray(inputs['bv'], np.float32)
    bo = np.asarray(inputs['bo'], np.float32)
    y += (Wo @ bv + bo)[None, :]
    return y


def kernel(**inputs) -> np.ndarray:
    in_maps = build_in_maps(inputs)
    nc = get_nc()
    res = run_bass_kernel_spmd(nc, in_maps, core_ids=list(range(N_CORES)))
    return gather(res.results, inputs)


# revision 20
# speedup vs baseline: 1.4558x; 1.4558x over previous
"""Multi-head causal self-attention (SEQ=4096, D=1024, H=16, Dh=64) on 8
Trainium2 NeuronCores.

Sharding: tensor-parallel over heads - 2 heads per core. Each core computes
its heads' Q/K/V projections, causal attention, and its partial output
projection Y_c = O_c @ Wo[:, c]^T. The 8 partials are summed on the host
(mathematically the all-reduce); bo and the V-bias correction Wo@bv are
added there.

Device kernel, fully software-pipelined (matmuls bf16, fp32 PSUM):
  - one loop over 8 q-blocks (QB=512); during block qb's attention k-loop
    the kernel interleaves: QK projections + V projection of block qb+1,
    softmax-normalization of block qb-1, and the output projection of
    block qb-1. No separate phases, so PE and ACT overlap end to end.
  - S^T = K Q^T per (q-block, k-tile): K stationaries are zero-padded to
    128 contraction rows (KTz0 = [K0;0], KTz1 = [0;K1]) so every matmul
    runs in the untiled 128x128 PE mode - no 64-row-mode switches, which
    cost a ~130-160ns array drain per switch.
  - V is projected with x-chunk stationaries (out = x_tile^T @ Wv chunk,
    [128 seq, 128 hp]), which directly yields the AV-stationary k-major
    layout - no PE transposes (transpose mode switches) anywhere.
  - AV: stationary [V_h | ones] (M=65) so the matmul also accumulates the
    softmax row-sums; normalization = DVE reciprocal + gpsimd
    partition_broadcast + DVE multiplies (ACT runs exp only, one table).
  - diagonal k-tiles are trimmed: S/exp/mask/AV only touch columns
    q >= 128*j, saving PE cycles, exp elements and gpsimd mask work.
  - bk is dropped (softmax shift invariance), bv/bo folded into a host
    constant Wo@bv+bo, y is stored fp16 (halves the writeback traffic).

The causal mask input is not read: the reference mask is tril(ones) by
construction and the kernel hardcodes causality.
"""
import sys

if '/opt/trn_rl_repo' not in sys.path:
    sys.path.insert(0, '/opt/trn_rl_repo')

import numpy as np

import concourse.bass as bass
import concourse.mybir as mybir
import concourse.tile as tile
from concourse.bass_utils import run_bass_kernel_spmd

SEQ = 4096
D = 1024
N_CORES = 8
HP = 128          # head dims per core (2 heads x 64)
DH = 64
QB = 512          # q-block
KB = 128          # k-tile
NQB = SEQ // QB   # 8
NKT = SEQ // KB   # 32
NDC = D // 128    # 8 contraction chunks for the projections

F32 = mybir.dt.float32
F16 = mybir.dt.float16
BF16 = mybir.dt.bfloat16

_NC_CACHE = None


def _split_waits(nc):
    """This walrus build allows only one sync-wait per instruction for
    several ISA structs (self-loading matmuls, drains, DMAs, DVE ops).
    Offload extra waits onto single-wait EventSemaphores inserted
    immediately before, on the same engine."""
    n = 0
    for f in nc.m.functions:
        for b in f.blocks:
            insts = b.instructions  # live list
            i = 0
            while i < len(insts):
                inst = insts[i]
                tn = type(inst).__name__
                if tn != 'InstEventSemaphore':
                    si = inst.sync_info
                    waits = list(si.on_wait) if si and si.on_wait else []
                    if len(waits) > 1:
                        for j, w in enumerate(waits[:-1]):
                            ev = mybir.InstEventSemaphore(
                                name=f'mmwait-{n}-{j}-{inst.name}',
                                engine=inst.engine,
                                ins=[], outs=[],
                                sync_info=mybir.SyncInfo(
                                    on_wait=[w], on_update=[]),
                            )
                            insts.insert(i, ev)
                            i += 1
                        inst.sync_info = mybir.SyncInfo(
                            on_wait=[waits[-1]],
                            on_update=list(si.on_update or []))
                        n += 1
                i += 1
    return n


def _build_nc():
    nc = bass.Bass()
    # x pre-chunked and pre-cast to bf16 on host:
    # [qc, p, c, q] = x[qc*QB+q, c*128+p]
    xT = nc.dram_tensor('xT', [NQB, 128, NDC, QB], BF16, kind='ExternalInput')
    # W pre-chunked, bf16: [p, c, m] = W.T[c*128+p, m]
    wqT = nc.dram_tensor('wqT', [128, NDC, HP], BF16, kind='ExternalInput')
    wkT = nc.dram_tensor('wkT', [128, NDC, HP], BF16, kind='ExternalInput')
    wvT = nc.dram_tensor('wvT', [128, NDC, HP], BF16, kind='ExternalInput')
    bq = nc.dram_tensor('bq', [HP, 1], F32, kind='ExternalInput')
    woT = nc.dram_tensor('woT', [HP, D], BF16, kind='ExternalInput')
    y = nc.dram_tensor('y', [SEQ, D], F16, kind='ExternalOutput')

    ydma_engines = None  # set inside

    with tile.TileContext(nc) as tc:
        with tc.tile_pool(name='persist', bufs=1) as persist, \
             tc.tile_pool(name='dram', bufs=1, space='DRAM') as dpool, \
             tc.tile_pool(name='xb', bufs=3) as xbpool, \
             tc.tile_pool(name='ps', bufs=1, space='PSUM') as ps, \
             tc.tile_pool(name='p01s', bufs=4) as ppool, \
             tc.tile_pool(name='rbp', bufs=2) as rbpool, \
             tc.tile_pool(name='yp', bufs=3) as ypool:

            ydma_engines = [nc.sync]

            def load_chunk(qc, nsplit=2):
                xb = xbpool.tile([128, NDC, QB], BF16, tag='xb')
                step = NDC // nsplit
                for a in range(nsplit):
                    csl = bass.ts(a, step)
                    nc.sync.dma_start(out=xb[:, csl, :],
                                      in_=xT[qc, :, csl, :])
                return xb

            # ---- persistent SBUF state ----
            bq_sb = persist.tile([HP, 1], F32)
            nc.gpsimd.dma_start(out=bq_sb, in_=bq[:, :])
            wq_b = persist.tile([128, NDC, HP], BF16)
            wk_b = persist.tile([128, NDC, HP], BF16)
            wv_b = persist.tile([128, NDC, HP], BF16)
            wo_b = persist.tile([HP, D], BF16)
            for dram_w, btile in ((wqT, wq_b), (wkT, wk_b), (wvT, wv_b)):
                nc.gpsimd.dma_start(out=btile, in_=dram_w[:, :, :])
            nc.gpsimd.dma_start(out=wo_b, in_=woT[:, :])

            QT = persist.tile([HP, SEQ], BF16)
            KTz0 = persist.tile([128, SEQ], BF16)  # [K0; 0]
            KTz1 = persist.tile([128, SEQ], BF16)  # [0; K1]
            V_sb = persist.tile([128, NKT, 130], BF16)  # [k, kt, V0|1|V1|1]
            OT = persist.tile([HP, SEQ], BF16)
            rs_row = persist.tile([1, NQB, 2, QB], F32)
            rs_dr = dpool.tile([1, NQB, 2, QB], F32)
            rc_dr = dpool.tile([1, NQB, 2, QB], F32)

            # zero the dead halves of the padded K stationaries, set the
            # AV ones-columns (both written once; projections never touch
            # them again)
            nc.vector.memset(KTz0[64:128, :], 0.0)
            nc.vector.memset(KTz1[0:64, :], 0.0)
            nc.vector.memset(V_sb[:, :, 64:65], 1.0)
            nc.vector.memset(V_sb[:, :, 129:130], 1.0)

            xbufs = {}
            xbufs[0] = load_chunk(0, nsplit=8)
            xbufs[1] = load_chunk(1, nsplit=2)

            # ---- PE warmup during the first x DMA ----
            warm = ps.tile([128, 128], F32, tag='work')
            for i in range(48):
                nc.tensor.matmul(warm[0:HP, :], wq_b[:, 0, :],
                                 wk_b[:, 0, :], start=(i == 0),
                                 stop=(i == 47))

            # ---- building blocks ----
            def qk_mm_task(qkt_ps, xb, qc, d):
                st = (d == 0)
                sp = (d == NDC - 1)
                nc.tensor.matmul(qkt_ps[:, 0, :], wq_b[:, d, :],
                                 xb[:, d, :], start=st, stop=sp)
                nc.tensor.matmul(qkt_ps[:, 1, :], wk_b[:, d, :],
                                 xb[:, d, :], start=st, stop=sp)

            def qk_copy_task(qkt_ps, qc):
                qsl = bass.ts(qc, QB)
                nc.vector.tensor_scalar_add(QT[:, qsl], qkt_ps[:, 0, :],
                                            bq_sb[:, 0:1])
                nc.vector.tensor_copy(out=KTz0[0:64, qsl],
                                      in_=qkt_ps[0:64, 1, :])
                nc.vector.tensor_copy(out=KTz1[64:128, qsl],
                                      in_=qkt_ps[64:128, 1, :])

            def v_mm_task(v_ps, xb, qc, j, d):
                nc.tensor.matmul(v_ps[:, j, :], xb[:, d, bass.ts(j, 128)],
                                 wv_b[:, d, :], start=(d == 0),
                                 stop=(d == NDC - 1))

            def v_copy_task(v_ps, qc, j):
                kt = qc * (QB // 128) + j
                nc.vector.tensor_copy(out=V_sb[:, kt, 0:DH],
                                      in_=v_ps[:, j, 0:DH])
                nc.vector.tensor_copy(out=V_sb[:, kt, 65:65 + DH],
                                      in_=v_ps[:, j, DH:2 * DH])

            def norm_a(qb, o01):
                # phase A, at the next block's start: move the row-sums
                # (o01 row 64, the AV ones column) and the unnormalized
                # O^T out of PSUM so o01's slot frees quickly, and compute
                # the reciprocals in a partition-parallel [128, 2, 4]
                # layout via a DRAM scatter round-trip (a single-partition
                # DVE reciprocal takes 6.5us; this one is ~0.2us).
                qsl = bass.ts(qb, QB)
                nc.vector.tensor_copy(out=rs_row[0:1, qb, :, :],
                                      in_=o01[64:65, :, :])
                nc.vector.tensor_copy(out=OT[0:DH, qsl],
                                      in_=o01[0:DH, 0, :])
                nc.vector.tensor_copy(out=OT[DH:2 * DH, qsl],
                                      in_=o01[0:DH, 1, :])
                nc.sync.dma_start(out=rs_dr[0:1, qb, :, :],
                                  in_=rs_row[0:1, qb, :, :])
                # contiguous 8-element (32B) lines per partition so each
                # round-trip is one descriptor, not a per-element scatter
                rs_t = rbpool.tile([128, 8], F32, tag='rst')
                nc.sync.dma_start(
                    out=rs_t,
                    in_=bass.AP(tensor=rs_dr.tensor,
                                offset=rs_dr.offset + 2 * qb * QB,
                                ap=[[8, 128], [1, 8]]))
                rc_t = rbpool.tile([128, 8], F32, tag='rct')
                nc.vector.reciprocal(out=rc_t, in_=rs_t)
                nc.sync.dma_start(
                    out=bass.AP(tensor=rc_dr.tensor,
                                offset=rc_dr.offset + 2 * qb * QB,
                                ap=[[8, 128], [1, 8]]),
                    in_=rc_t)

            def norm_b(qb):
                # phase B, an interleaved task late in the next block:
                # broadcast the reciprocals across partitions (DMA read
                # with partition stride 0) and scale OT in place. Runs on
                # gpsimd+DVE only, after the block's affine_selects are
                # already enqueued.
                qsl = bass.ts(qb, QB)
                rb = rbpool.tile([128, QB], F32, tag='rb')
                for h in range(2):
                    # on sync, not gpsimd: the gpsimd FIFO must stay clear
                    # for the attention masks (affine_select)
                    nc.sync.dma_start(
                        out=rb[h * DH:(h + 1) * DH, :],
                        in_=bass.AP(tensor=rc_dr.tensor,
                                    offset=rc_dr.offset + (2 * qb + h) * QB,
                                    ap=[[0, DH], [1, QB]]))
                nc.vector.tensor_mul(OT[0:DH, qsl],
                                     OT[0:DH, qsl], rb[0:DH, :])
                nc.vector.tensor_mul(OT[DH:2 * DH, qsl],
                                     OT[DH:2 * DH, qsl], rb[DH:2 * DH, :])

            ycount = [0]

            def yproj_task(t, tag='work'):
                # output projection of q-tile t (128 rows), from OT
                qt_sl = bass.ts(t, 128)
                y01 = ps.tile([128, 2, QB], F32, tag=tag,
                              bufs=2 if tag == 's01' else None)
                nc.tensor.matmul(y01[:, 0, :], OT[:, qt_sl],
                                 wo_b[:, 0:QB], start=True, stop=True)
                nc.tensor.matmul(y01[:, 1, :], OT[:, qt_sl],
                                 wo_b[:, QB:D], start=True, stop=True)
                ysb = ypool.tile([128, D], F16)
                nc.vector.tensor_copy(out=ysb,
                                      in_=y01.rearrange('p a b -> p (a b)'))
                eng = ydma_engines[ycount[0] % len(ydma_engines)]
                ycount[0] += 1
                eng.dma_start(out=y[qt_sl, :], in_=ysb)

            # ---- block-qb work queue (interleaved into the k-loop) ----
            def make_work(qb):
                tasks = []
                pj = qb + 1
                if pj < NQB:
                    qkt_ps = ps.tile([128, 2, QB], F32, tag='work',
                                     name=f'qkt{pj}')
                    xb = xbufs[pj]
                    for d in range(NDC):
                        tasks.append(lambda q=pj, dd=d, t=qkt_ps, x=xb:
                                     qk_mm_task(t, x, q, dd))
                    tasks.append(lambda q=pj, t=qkt_ps: qk_copy_task(t, q))
                    v_ps = ps.tile([128, 4, 128], F32, tag='work',
                                   name=f'vps{pj}')
                    for j in range(4):
                        for d in range(NDC):
                            tasks.append(lambda q=pj, jj=j, dd=d, t=v_ps,
                                         x=xb: v_mm_task(t, x, q, jj, dd))
                        tasks.append(lambda q=pj, jj=j, t=v_ps:
                                     v_copy_task(t, q, jj))
                    if pj + 1 < NQB:
                        # prefetch x for the next projection block
                        tasks.append(lambda q=pj + 1:
                                     xbufs.__setitem__(q, load_chunk(q)))
                if qb >= 1:
                    tasks.append(lambda b=qb - 1: norm_b(b))
                    for t in range((qb - 1) * 4, qb * 4):
                        tasks.append(lambda tt=t: yproj_task(tt))
                return tasks

            # ---- projection of block 0 (bootstrap, d-chunk streamed) ----
            qkt0 = ps.tile([128, 2, QB], F32, tag='work')
            for d in range(NDC):
                qk_mm_task(qkt0, xbufs[0], 0, d)
            qk_copy_task(qkt0, 0)
            v0 = ps.tile([128, 4, 128], F32, tag='work')
            for j in range(4):
                for d in range(NDC):
                    v_mm_task(v0, xbufs[0], 0, j, d)
                v_copy_task(v0, 0, j)

            # ---- main loop: attention(qb) + interleaved work ----
            prev_o01 = None
            for qb in range(NQB):
                qsl = bass.ts(qb, QB)
                nsteps = (qb + 1) * (QB // KB)
                diag0 = qb * (QB // KB)
                if qb >= 1:
                    # must precede the o01 reallocation below: the slot is
                    # single-buffered, so the norm reads of block qb-1 have
                    # to be in program order before block qb's first AV
                    norm_a(qb - 1, prev_o01)
                o01 = ps.tile([65, 2, QB], F32, tag='o01')
                tasks = make_work(qb)
                prev_o01 = o01
                # spread tasks over steps (front-loaded so proj finishes
                # before the next block needs it)
                ntask = len(tasks)
                emitted = 0
                pending_av = None
                for step in range(nsteps):
                    kt = step
                    j = kt - diag0
                    off = 128 * j if j >= 1 else 0
                    q0 = qb * QB + off
                    q1 = (qb + 1) * QB
                    s01 = ps.tile([128, 2, QB], F32, tag='s01', bufs=2)
                    ksl = bass.ts(kt, KB)
                    nc.tensor.matmul(s01[:, 0, off:], KTz0[:, ksl],
                                     QT[:, q0:q1], start=True, stop=True)
                    nc.tensor.matmul(s01[:, 1, off:], KTz1[:, ksl],
                                     QT[:, q0:q1], start=True, stop=True)
                    p01 = ppool.tile([128, 2, QB], BF16, tag='p01')
                    nc.scalar.activation(
                        out=p01[:, :, off:], in_=s01[:, :, off:],
                        func=mybir.ActivationFunctionType.Exp)
                    if j >= 0:
                        # diagonal tile: zero entries with k > q in the
                        # first 128 valid columns; later columns are
                        # always unmasked
                        nc.gpsimd.affine_select(
                            out=p01[:, :, off:off + 128],
                            in_=p01[:, :, off:off + 128],
                            compare_op=mybir.AluOpType.is_ge,
                            fill=0.0, base=0,
                            pattern=[[0, 2], [1, 128]],
                            channel_multiplier=-1)
                    # AV of the previous step: one step of lag keeps the
                    # in-order PE queue from stalling on exp(n)
                    if pending_av is not None:
                        pending_av()
                    st = (step == 0)
                    sp = (step == nsteps - 1)

                    def av(o=off, k=kt, p=p01, st=st, sp=sp):
                        nc.tensor.matmul(o01[:, 0, o:], V_sb[:, k, 0:65],
                                         p[:, 0, o:], start=st, stop=sp,
                                         skip_group_check=True)
                        nc.tensor.matmul(o01[:, 1, o:], V_sb[:, k, 65:130],
                                         p[:, 1, o:], start=st, stop=sp,
                                         skip_group_check=True)
                    pending_av = av
                    # interleave background work
                    want = (step + 1) * ntask // nsteps
                    while emitted < want:
                        tasks[emitted]()
                        emitted += 1
                pending_av()

            # ---- tail: last block's norm + output projection ----
            norm_a(NQB - 1, prev_o01)
            norm_b(NQB - 1)
            for t in range((NQB - 1) * 4, NQB * 4):
                yproj_task(t, tag='s01')

    _split_waits(nc)
    return nc


def get_nc():
    global _NC_CACHE
    if _NC_CACHE is None:
        _NC_CACHE = _build_nc()
    return _NC_CACHE


def _chunk_w(wT):
    # [D, HP] -> [p, c, m] with D = c*128 + p, cast to bf16
    import ml_dtypes
    return np.ascontiguousarray(
        wT.reshape(NDC, 128, HP).transpose(1, 0, 2)).astype(
            ml_dtypes.bfloat16)


def build_in_maps(inputs):
    import ml_dtypes
    x = np.asarray(inputs['x'], np.float32)
    # [qc, p, c, q] = x[qc*QB+q, c*128+p], bf16
    xc = np.ascontiguousarray(
        x.reshape(NQB, QB, NDC, 128).transpose(0, 3, 2, 1)).astype(
            ml_dtypes.bfloat16)
    scale = 1.0 / np.sqrt(DH)
    Wq = np.asarray(inputs['Wq'], np.float32)
    Wk = np.asarray(inputs['Wk'], np.float32)
    Wv = np.asarray(inputs['Wv'], np.float32)
    Wo = np.asarray(inputs['Wo'], np.float32)
    bq = np.asarray(inputs['bq'], np.float32)
    in_maps = []
    for c in range(N_CORES):
        sl = slice(c * HP, (c + 1) * HP)
        in_maps.append({
            'xT': xc,
            'wqT': _chunk_w((Wq[sl, :] * scale).T),
            'wkT': _chunk_w(Wk[sl, :].T),
            'wvT': _chunk_w(Wv[sl, :].T),
            'bq': np.ascontiguousarray((bq[sl] * scale).reshape(HP, 1)),
            'woT': np.ascontiguousarray(Wo[:, sl].T).astype(
                ml_dtypes.bfloat16),
        })
    return in_maps


def gather(results, inputs):
    y = np.zeros((SEQ, D), np.float32)
    for r in results:
        y += np.asarray(r['y'], np.float32)
    Wo = np.asarray(inputs['Wo'], np.float32)
    bv = np.asarray(inputs['bv'], np.float32)
    bo = np.asarray(inputs['bo'], np.float32)
    y += (Wo @ bv + bo)[None, :]
    return y


def kernel(**inputs) -> np.ndarray:
    in_maps = build_in_maps(inputs)
    nc = get_nc()
    res = run_bass_kernel_spmd(nc, in_maps, core_ids=list(range(N_CORES)))
    return gather(res.results, inputs)


# revision 30
# speedup vs baseline: 1.5354x; 1.0546x over previous
"""Multi-head causal self-attention (SEQ=4096, D=1024, H=16, Dh=64) on 8
Trainium2 NeuronCores.

Sharding: tensor-parallel over heads - 2 heads per core. Each core computes
its heads' Q/K/V projections, causal attention, and its partial output
projection Y_c = O_c @ Wo[:, c]^T. The 8 partials are summed on the host
(mathematically the all-reduce); bo and the V-bias correction Wo@bv are
added there.

Device kernel, fully software-pipelined (matmuls bf16, fp32 PSUM):
  - one loop over 8 q-blocks (QB=512); during block qb's attention k-loop
    the kernel interleaves: QK projections + V projection of block qb+1,
    softmax-normalization of block qb-1, and the output projection of
    block qb-1. No separate phases, so PE and ACT overlap end to end.
  - S^T = K Q^T per (q-block, k-tile): K stationaries are zero-padded to
    128 contraction rows (KTz0 = [K0;0], KTz1 = [0;K1]) so every matmul
    runs in the untiled 128x128 PE mode - no 64-row-mode switches, which
    cost a ~130-160ns array drain per switch.
  - V is projected with x-chunk stationaries (out = x_tile^T @ Wv chunk,
    [128 seq, 128 hp]), which directly yields the AV-stationary k-major
    layout - no PE transposes (transpose mode switches) anywhere.
  - AV: stationary [V_h | ones] (M=65) so the matmul also accumulates the
    softmax row-sums; normalization = DVE reciprocal + gpsimd
    partition_broadcast + DVE multiplies (ACT runs exp only, one table).
  - diagonal k-tiles are trimmed: S/exp/mask/AV only touch columns
    q >= 128*j, saving PE cycles, exp elements and gpsimd mask work.
  - bk is dropped (softmax shift invariance), bv/bo folded into a host
    constant Wo@bv+bo, y is stored fp16 (halves the writeback traffic).

The causal mask input is not read: the reference mask is tril(ones) by
construction and the kernel hardcodes causality.
"""
import sys

if '/opt/trn_rl_repo' not in sys.path:
    sys.path.insert(0, '/opt/trn_rl_repo')

import numpy as np

import concourse.bass as bass
import concourse.mybir as mybir
import concourse.tile as tile
from concourse.bass_utils import run_bass_kernel_spmd

SEQ = 4096
D = 1024
N_CORES = 8
HP = 128          # head dims per core (2 heads x 64)
DH = 64
QB = 512          # q-block
KB = 128          # k-tile
NQB = SEQ // QB   # 8
NKT = SEQ // KB   # 32
NDC = D // 128    # 8 contraction chunks for the projections

F32 = mybir.dt.float32
F16 = mybir.dt.float16
BF16 = mybir.dt.bfloat16

_NC_CACHE = None


def _split_waits(nc):
    """This walrus build allows only one sync-wait per instruction for
    several ISA structs (self-loading matmuls, drains, DMAs, DVE ops).
    Offload extra waits onto single-wait EventSemaphores inserted
    immediately before, on the same engine."""
    n = 0
    for f in nc.m.functions:
        for b in f.blocks:
            insts = b.instructions  # live list
            i = 0
            while i < len(insts):
                inst = insts[i]
                tn = type(inst).__name__
                if tn != 'InstEventSemaphore':
                    si = inst.sync_info
                    waits = list(si.on_wait) if si and si.on_wait else []
                    if len(waits) > 1:
                        for j, w in enumerate(waits[:-1]):
                            ev = mybir.InstEventSemaphore(
                                name=f'mmwait-{n}-{j}-{inst.name}',
                                engine=inst.engine,
                                ins=[], outs=[],
                                sync_info=mybir.SyncInfo(
                                    on_wait=[w], on_update=[]),
                            )
                            insts.insert(i, ev)
                            i += 1
                        inst.sync_info = mybir.SyncInfo(
                            on_wait=[waits[-1]],
                            on_update=list(si.on_update or []))
                        n += 1
                i += 1
    return n


def _build_nc():
    nc = bass.Bass()
    # x pre-chunked and pre-cast to bf16 on host:
    # [qc, p, c, q] = x[qc*QB+q, c*128+p]
    xT = nc.dram_tensor('xT', [NQB, 128, NDC, QB], BF16, kind='ExternalInput')
    # W pre-chunked, bf16: [p, c, m] = W.T[c*128+p, m]
    wqT = nc.dram_tensor('wqT', [128, NDC, HP], BF16, kind='ExternalInput')
    wkT = nc.dram_tensor('wkT', [128, NDC, HP], BF16, kind='ExternalInput')
    wvT = nc.dram_tensor('wvT', [128, NDC, HP], BF16, kind='ExternalInput')
    bq = nc.dram_tensor('bq', [HP, 1], F32, kind='ExternalInput')
    woT = nc.dram_tensor('woT', [HP, D], BF16, kind='ExternalInput')
    y = nc.dram_tensor('y', [SEQ, D], F16, kind='ExternalOutput')

    ydma_engines = None  # set inside

    with tile.TileContext(nc) as tc:
        with tc.tile_pool(name='persist', bufs=1) as persist, \
             tc.tile_pool(name='dram', bufs=1, space='DRAM') as dpool, \
             tc.tile_pool(name='xb', bufs=3) as xbpool, \
             tc.tile_pool(name='ps', bufs=1, space='PSUM') as ps, \
             tc.tile_pool(name='p01s', bufs=6) as ppool, \
             tc.tile_pool(name='rbp', bufs=2) as rbpool, \
             tc.tile_pool(name='yp', bufs=3) as ypool:

            ydma_engines = [nc.sync]

            def load_chunk(qc, nsplit=2):
                xb = xbpool.tile([128, NDC, QB], BF16, tag='xb')
                step = NDC // nsplit
                for a in range(nsplit):
                    csl = bass.ts(a, step)
                    nc.sync.dma_start(out=xb[:, csl, :],
                                      in_=xT[qc, :, csl, :])
                return xb

            # ---- persistent SBUF state ----
            bq_sb = persist.tile([HP, 1], F32)
            nc.gpsimd.dma_start(out=bq_sb, in_=bq[:, :])
            wq_b = persist.tile([128, NDC, HP], BF16)
            wk_b = persist.tile([128, NDC, HP], BF16)
            wv_b = persist.tile([128, NDC, HP], BF16)
            wo_b = persist.tile([HP, D], BF16)
            for dram_w, btile in ((wqT, wq_b), (wkT, wk_b), (wvT, wv_b)):
                nc.gpsimd.dma_start(out=btile, in_=dram_w[:, :, :])
            nc.gpsimd.dma_start(out=wo_b, in_=woT[:, :])

            QT = persist.tile([HP, SEQ], BF16)
            KT = persist.tile([HP, SEQ], BF16)
            V_sb = persist.tile([128, NKT, 130], BF16)  # [k, kt, V0|1|V1|1]
            OT = persist.tile([HP, SEQ], BF16)
            rs_row = persist.tile([1, NQB, 2, QB], F32)
            rs_dr = dpool.tile([1, NQB, 2, QB], F32)
            rc_dr = dpool.tile([1, NQB, 2, QB], F32)

            # the AV ones-columns, written once; projections never touch
            # them again
            nc.vector.memset(V_sb[:, :, 64:65], 1.0)
            nc.vector.memset(V_sb[:, :, 129:130], 1.0)
            # preload the exp activation table off the critical path
            tbl_warm = persist.tile([1, 1], F32)
            nc.vector.memset(tbl_warm, 0.0)
            nc.scalar.activation(out=tbl_warm, in_=tbl_warm,
                                 func=mybir.ActivationFunctionType.Exp)

            xbufs = {}
            xbufs[0] = load_chunk(0, nsplit=8)
            xbufs[1] = load_chunk(1, nsplit=2)

            # ---- PE warmup during the first x DMA ----
            warm = ps.tile([128, 128], F32, tag='work')
            for i in range(24):
                nc.tensor.matmul(warm[0:HP, :], wq_b[:, 0, :],
                                 wk_b[:, 0, :], start=(i == 0),
                                 stop=(i == 23))

            # ---- building blocks ----
            def qk_mm_task(qkt_ps, xb, qc, d):
                st = (d == 0)
                sp = (d == NDC - 1)
                nc.tensor.matmul(qkt_ps[:, 0, :], wq_b[:, d, :],
                                 xb[:, d, :], start=st, stop=sp)
                nc.tensor.matmul(qkt_ps[:, 1, :], wk_b[:, d, :],
                                 xb[:, d, :], start=st, stop=sp)

            def qk_copy_task(qkt_ps, qc):
                qsl = bass.ts(qc, QB)
                nc.vector.tensor_scalar_add(QT[:, qsl], qkt_ps[:, 0, :],
                                            bq_sb[:, 0:1])
                nc.vector.tensor_copy(out=KT[:, qsl], in_=qkt_ps[:, 1, :])

            def v_mm_task(v_ps, xb, qc, j, d):
                nc.tensor.matmul(v_ps[:, j, :], xb[:, d, bass.ts(j, 128)],
                                 wv_b[:, d, :], start=(d == 0),
                                 stop=(d == NDC - 1))

            def v_copy_task(v_ps, qc, j):
                kt = qc * (QB // 128) + j
                nc.vector.tensor_copy(out=V_sb[:, kt, 0:DH],
                                      in_=v_ps[:, j, 0:DH])
                nc.vector.tensor_copy(out=V_sb[:, kt, 65:65 + DH],
                                      in_=v_ps[:, j, DH:2 * DH])

            def norm_a(qb, o01, eng=None):
                eng = eng or nc.sync
                # phase A, at the next block's start: move the row-sums
                # (o01 row 64, the AV ones column) and the unnormalized
                # O^T out of PSUM so o01's slot frees quickly, and compute
                # the reciprocals in a partition-parallel [128, 2, 4]
                # layout via a DRAM scatter round-trip (a single-partition
                # DVE reciprocal takes 6.5us; this one is ~0.2us).
                qsl = bass.ts(qb, QB)
                nc.vector.tensor_copy(out=rs_row[0:1, qb, :, :],
                                      in_=o01[64:65, :, :])
                nc.vector.tensor_copy(out=OT[0:DH, qsl],
                                      in_=o01[0:DH, 0, :])
                nc.vector.tensor_copy(out=OT[DH:2 * DH, qsl],
                                      in_=o01[0:DH, 1, :])
                eng.dma_start(out=rs_dr[0:1, qb, :, :],
                                  in_=rs_row[0:1, qb, :, :])
                # contiguous 8-element (32B) lines per partition so each
                # round-trip is one descriptor, not a per-element scatter
                rs_t = rbpool.tile([128, 8], F32, tag='rst')
                eng.dma_start(
                    out=rs_t,
                    in_=bass.AP(tensor=rs_dr.tensor,
                                offset=rs_dr.offset + 2 * qb * QB,
                                ap=[[8, 128], [1, 8]]))
                rc_t = rbpool.tile([128, 8], F32, tag='rct')
                nc.vector.reciprocal(out=rc_t, in_=rs_t)
                eng.dma_start(
                    out=bass.AP(tensor=rc_dr.tensor,
                                offset=rc_dr.offset + 2 * qb * QB,
                                ap=[[8, 128], [1, 8]]),
                    in_=rc_t)

            def norm_b(qb, eng=None):
                eng = eng or nc.sync
                # phase B, an interleaved task late in the next block:
                # broadcast the reciprocals across partitions (DMA read
                # with partition stride 0) and scale OT in place. Runs on
                # gpsimd+DVE only, after the block's affine_selects are
                # already enqueued.
                qsl = bass.ts(qb, QB)
                rb = rbpool.tile([128, QB], F32, tag='rb')
                for h in range(2):
                    # on sync, not gpsimd: the gpsimd FIFO must stay clear
                    # for the attention masks (affine_select)
                    eng.dma_start(
                        out=rb[h * DH:(h + 1) * DH, :],
                        in_=bass.AP(tensor=rc_dr.tensor,
                                    offset=rc_dr.offset + (2 * qb + h) * QB,
                                    ap=[[0, DH], [1, QB]]))
                nc.vector.tensor_mul(OT[0:DH, qsl],
                                     OT[0:DH, qsl], rb[0:DH, :])
                nc.vector.tensor_mul(OT[DH:2 * DH, qsl],
                                     OT[DH:2 * DH, qsl], rb[DH:2 * DH, :])

            ycount = [0]

            def yproj_task(t, tag='work', eng=None):
                # output projection of q-tile t (128 rows), from OT
                qt_sl = bass.ts(t, 128)
                y01 = ps.tile([128, 2, QB], F32, tag=tag,
                              bufs=2 if tag == 's01' else None)
                nc.tensor.matmul(y01[:, 0, :], OT[:, qt_sl],
                                 wo_b[:, 0:QB], start=True, stop=True)
                nc.tensor.matmul(y01[:, 1, :], OT[:, qt_sl],
                                 wo_b[:, QB:D], start=True, stop=True)
                ysb = ypool.tile([128, D], F16)
                nc.vector.tensor_copy(out=ysb,
                                      in_=y01.rearrange('p a b -> p (a b)'))
                if eng is None:
                    eng = ydma_engines[ycount[0] % len(ydma_engines)]
                    ycount[0] += 1
                eng.dma_start(out=y[qt_sl, :], in_=ysb)

            # ---- block-qb work queue (interleaved into the k-loop) ----
            def make_work(qb):
                tasks = []
                pj = qb + 1
                if pj < NQB:
                    qkt_ps = ps.tile([128, 2, QB], F32, tag='work',
                                     name=f'qkt{pj}')
                    xb = xbufs[pj]
                    for d in range(NDC):
                        tasks.append(lambda q=pj, dd=d, t=qkt_ps, x=xb:
                                     qk_mm_task(t, x, q, dd))
                    tasks.append(lambda q=pj, t=qkt_ps: qk_copy_task(t, q))
                    v_ps = ps.tile([128, 4, 128], F32, tag='work',
                                   name=f'vps{pj}')
                    for j in range(4):
                        for d in range(NDC):
                            tasks.append(lambda q=pj, jj=j, dd=d, t=v_ps,
                                         x=xb: v_mm_task(t, x, q, jj, dd))
                        tasks.append(lambda q=pj, jj=j, t=v_ps:
                                     v_copy_task(t, q, jj))
                    if pj + 1 < NQB:
                        # prefetch x for the next projection block
                        tasks.append(lambda q=pj + 1:
                                     xbufs.__setitem__(q, load_chunk(q)))
                if qb >= 1:
                    tasks.append(lambda b=qb - 1: norm_b(b))
                    for t in range((qb - 1) * 4, qb * 4):
                        tasks.append(lambda tt=t: yproj_task(tt))
                return tasks

            # ---- projection of block 0 (bootstrap, d-chunk streamed) ----
            # only Q/K and the first V k-tile are done up front; V k-tiles
            # 1-3 go at the head of block 0's task list so the exp stream
            # starts as early as possible
            qkt0 = ps.tile([128, 2, QB], F32, tag='work')
            for d in range(NDC):
                qk_mm_task(qkt0, xbufs[0], 0, d)
            qk_copy_task(qkt0, 0)
            v0 = ps.tile([128, 4, 128], F32, tag='work')
            for d in range(NDC):
                v_mm_task(v0, xbufs[0], 0, 0, d)
            v_copy_task(v0, 0, 0)

            def v0_tail_tasks():
                tasks = []
                for j in range(1, 4):
                    for d in range(NDC):
                        tasks.append(lambda jj=j, dd=d:
                                     v_mm_task(v0, xbufs[0], 0, jj, dd))
                    tasks.append(lambda jj=j: v_copy_task(v0, 0, jj))
                return tasks

            # ---- main loop: attention(qb) + interleaved work ----
            prev_o01 = None
            for qb in range(NQB):
                qsl = bass.ts(qb, QB)
                nsteps = (qb + 1) * (QB // KB)
                diag0 = qb * (QB // KB)
                if qb >= 1:
                    # must precede the o01 reallocation below: the slot is
                    # single-buffered, so the norm reads of block qb-1 have
                    # to be in program order before block qb's first AV
                    norm_a(qb - 1, prev_o01)
                o01 = ps.tile([65, 2, QB], F32, tag='o01')
                tasks = make_work(qb)
                if qb == 0:
                    tasks = v0_tail_tasks() + tasks
                prev_o01 = o01
                # spread tasks over steps (front-loaded so proj finishes
                # before the next block needs it)
                ntask = len(tasks)
                emitted = 0
                pending = []  # AV closures, flushed with two steps of lag

                def emit_step_s(step):
                    # S^T pair for one k-tile: the two heads' matmuls have
                    # 64-row stationaries at base partitions 0 and 64, so
                    # the PE runs them concurrently as two 64x128 tiles.
                    kt = step
                    j = kt - diag0
                    off = 128 * j if j >= 1 else 0
                    q0 = qb * QB + off
                    q1 = (qb + 1) * QB
                    s01 = ps.tile([128, 2, QB], F32, tag='s01', bufs=2)
                    ksl = bass.ts(kt, KB)
                    nc.tensor.matmul(s01[:, 0, off:], KT[0:DH, ksl],
                                     QT[0:DH, q0:q1], start=True, stop=True)
                    nc.tensor.matmul(s01[:, 1, off:], KT[DH:2 * DH, ksl],
                                     QT[DH:2 * DH, q0:q1],
                                     start=True, stop=True)
                    p01 = ppool.tile([128, 2, QB], BF16, tag='p01')
                    nc.scalar.activation(
                        out=p01[:, :, off:], in_=s01[:, :, off:],
                        func=mybir.ActivationFunctionType.Exp)
                    if j >= 0:
                        # diagonal tile: zero entries with k > q in the
                        # first 128 valid columns; later columns are
                        # always unmasked
                        nc.gpsimd.affine_select(
                            out=p01[:, :, off:off + 128],
                            in_=p01[:, :, off:off + 128],
                            compare_op=mybir.AluOpType.is_ge,
                            fill=0.0, base=0,
                            pattern=[[0, 2], [1, 128]],
                            channel_multiplier=-1)
                    st = (step == 0)
                    sp = (step == nsteps - 1)

                    def av(o=off, k=kt, p=p01, st=st, sp=sp):
                        nc.tensor.matmul(o01[:, 0, o:], V_sb[:, k, 0:65],
                                         p[:, 0, o:], start=st, stop=sp,
                                         skip_group_check=True)
                        nc.tensor.matmul(o01[:, 1, o:], V_sb[:, k, 65:130],
                                         p[:, 1, o:], start=st, stop=sp,
                                         skip_group_check=True)
                    pending.append(av)

                # steps run in pairs: both S pairs of the pair back to
                # back (one 64-row-mode burst, amortizing the PE mode
                # drain), then the AVs from two steps ago, then work
                for m in range(nsteps // 2):
                    emit_step_s(2 * m)
                    emit_step_s(2 * m + 1)
                    while len(pending) > 2:
                        pending.pop(0)()
                    want = (2 * m + 2) * ntask // nsteps
                    while emitted < want:
                        tasks[emitted]()
                        emitted += 1
                for av in pending:
                    av()
                pending.clear()

            # ---- tail: last block's norm + output projection ----
            norm_a(NQB - 1, prev_o01, eng=nc.scalar)
            norm_b(NQB - 1, eng=nc.scalar)
            for i, t in enumerate(range((NQB - 1) * 4, NQB * 4)):
                yproj_task(t, tag='s01',
                           eng=nc.scalar if i % 2 else nc.sync)

    _split_waits(nc)
    return nc


def get_nc():
    global _NC_CACHE
    if _NC_CACHE is None:
        _NC_CACHE = _build_nc()
    return _NC_CACHE


def _chunk_w(wT):
    # [D, HP] -> [p, c, m] with D = c*128 + p, cast to bf16
    import ml_dtypes
    return np.ascontiguousarray(
        wT.reshape(NDC, 128, HP).transpose(1, 0, 2)).astype(
            ml_dtypes.bfloat16)


def build_in_maps(inputs):
    import ml_dtypes
    x = np.asarray(inputs['x'], np.float32)
    # [qc, p, c, q] = x[qc*QB+q, c*128+p], bf16
    xc = np.ascontiguousarray(
        x.reshape(NQB, QB, NDC, 128).transpose(0, 3, 2, 1)).astype(
            ml_dtypes.bfloat16)
    scale = 1.0 / np.sqrt(DH)
    Wq = np.asarray(inputs['Wq'], np.float32)
    Wk = np.asarray(inputs['Wk'], np.float32)
    Wv = np.asarray(inputs['Wv'], np.float32)
    Wo = np.asarray(inputs['Wo'], np.float32)
    bq = np.asarray(inputs['bq'], np.float32)
    in_maps = []
    for c in range(N_CORES):
        sl = slice(c * HP, (c + 1) * HP)
        in_maps.append({
            'xT': xc,
            'wqT': _chunk_w((Wq[sl, :] * scale).T),
            'wkT': _chunk_w(Wk[sl, :].T),
            'wvT': _chunk_w(Wv[sl, :].T),
            'bq': np.ascontiguousarray((bq[sl] * scale).reshape(HP, 1)),
            'woT': np.ascontiguousarray(Wo[:, sl].T).astype(
                ml_dtypes.bfloat16),
        })
    return in_maps


def gather(results, inputs):
    y = np.zeros((SEQ, D), np.float32)
    for r in results:
        y += np.asarray(r['y'], np.float32)
    Wo = np.asarray(inputs['Wo'], np.float32)
    bv = np.asarray(inputs['bv'], np.float32)
    bo = np.asarray(inputs['bo'], np.float32)
    y += (Wo @ bv + bo)[None, :]
    return y


def kernel(**inputs) -> np.ndarray:
    in_maps = build_in_maps(inputs)
    nc = get_nc()
    res = run_bass_kernel_spmd(nc, in_maps, core_ids=list(range(N_CORES)))
    return gather(res.results, inputs)
